# revision 48
# baseline (speedup 1.0000x reference)
"""Trainium2 Bass kernel for nn_HHGR (gnn_message_passing).

Strategy (8 NeuronCores, groups sharded 1024/core):
  host prep: sum_tab = user_table + user_embedding (bf16), per-core shards
    of members/mask, H^T slabs (bf16, k-rows permuted to AllGather order),
    gtab^T, replicated small weights.
  device per core (1024 groups = 8 superblocks x 128 groups = 256 tiles):
    * gather member rows (bf16) - either 32 indirect DMAs per superblock
      (KGATHER=device) or one sequential DMA of host-pregathered slab
      (KGATHER=host)
    * attention, per superblock:
        PE-transpose emb tiles -> embT chunks; hid^T = aw1^T @ embT with
        4 chunks stacked in one PSUM bank (partition offsets 0/32/64/96);
        one batched Relu+bias; per-tile natural logits (lhsT = hidT slice,
        rhs = aw2); one batched Exp+bias -> p [128,32]; pm = p * mask;
        denominators via one matmul; 1/den broadcast to rows via 4 tiny
        K=1 matmuls; maskp = pm * recip; g_att^T accumulated per tile
        (lhsT = emb tile, rhs = maskp cols); X^T = g_att^T + gtab^T;
        V = X @ hw1 -> vdram half.
    * AllGather V in two halves (issued after superblock 3 and 7) so the
      first collective hides under attention of superblocks 4-7.
    * stage 1: Y^T = V^T H^T, k-tiles of half 1 accumulate while half 2's
      AllGather is in flight; relu -> h^T; W = h @ hw2 per column half,
      each followed by its W-AllGather; stage 2 (out^T = W^T H^T) k-tiles
      of half 1 overlap the second W-AllGather.
    * out^T [128, 1024] f32 written once; host transposes.
"""
import sys
sys.path.insert(0, "/opt/trn_rl_repo")

import os
import numpy as np
import ml_dtypes

import concourse.bass as bass
import concourse.bacc as bacc
import concourse.mybir as mybir
import concourse.tile as tile
from concourse.bass_utils import run_bass_kernel_spmd

F32 = mybir.dt.float32
BF16 = mybir.dt.bfloat16
I32 = mybir.dt.int32
AF = mybir.ActivationFunctionType

G, M, D, U = 8192, 32, 128, 200000
H_ATT = 16
NC = 8
GPC = G // NC          # 1024 groups per core
R = GPC * M            # 32768 member rows per core
NT = R // 128          # 256 tiles of 128 rows
NSB = GPC // 128       # 8 superblocks of 128 groups (32 tiles each)
KT = G // 128          # 64 k-tiles for the big matmuls
HGPC = GPC // 2        # half of the per-core groups (AllGather halves)

_CACHE = {}


def _build(gather="device"):
    nc = bacc.Bacc("TRN2", target_bir_lowering=False, debug=False)

    # ---- inputs ----
    if gather == "device":
        sum_tab = nc.dram_tensor("sum_tab", [U, D], BF16, kind="ExternalInput")
        gidx = nc.dram_tensor("gidx", [128, NT], I32, kind="ExternalInput")
    else:
        memb = nc.dram_tensor("memb", [128, NT * D], BF16, kind="ExternalInput")
    mdiag = nc.dram_tensor("mdiag", [128, NT * 4], BF16, kind="ExternalInput")
    gtabt = nc.dram_tensor("gtabt", [D, GPC], F32, kind="ExternalInput")
    ht = nc.dram_tensor("ht", [G, GPC], BF16, kind="ExternalInput")
    aw1 = nc.dram_tensor("aw1", [D, H_ATT], BF16, kind="ExternalInput")
    aw2r = nc.dram_tensor("aw2r", [128, 1], BF16, kind="ExternalInput")
    ab1s = nc.dram_tensor("ab1s", [128, 1], F32, kind="ExternalInput")
    ab2s = nc.dram_tensor("ab2s", [128, 1], F32, kind="ExternalInput")
    hw1 = nc.dram_tensor("hw1", [D, D], BF16, kind="ExternalInput")
    hw2 = nc.dram_tensor("hw2", [D, D], BF16, kind="ExternalInput")
    id_bf = nc.dram_tensor("id_bf", [128, 128], BF16, kind="ExternalInput")
    id_f32 = nc.dram_tensor("id_f32", [128, 128], F32, kind="ExternalInput")
    ind4 = nc.dram_tensor("ind4", [1, 512], F32, kind="ExternalInput")
    ones_col = nc.dram_tensor("ones_col", [128, 1], BF16, kind="ExternalInput")

    outt = nc.dram_tensor("outt", [D, GPC], F32, kind="ExternalOutput")

    # internal DRAM for collectives — per-core halves, stored transposed-
    # tiled [128, 512] so the post-AllGather reload is 1KB-contiguous per
    # partition: vd[h][p, j*128+d] = V[local group h*512 + j*128 + p, d]
    vd = [nc.dram_tensor(f"vd{h}", [128, HGPC], BF16) for h in range(2)]
    vag = [
        nc.dram_tensor(f"vag{h}", [NC * 128, HGPC], BF16, addr_space="Shared")
        for h in range(2)
    ]
    wd = [nc.dram_tensor(f"wd{h}", [128, HGPC], BF16) for h in range(2)]
    wag = [
        nc.dram_tensor(f"wag{h}", [NC * 128, HGPC], BF16, addr_space="Shared")
        for h in range(2)
    ]

    def allgather(src, dst):
        nc.gpsimd.collective_compute(
            "AllGather",
            mybir.AluOpType.bypass,
            replica_groups=[list(range(NC))],
            ins=[src[:]],
            outs=[dst[:]],
        )

    with tile.TileContext(nc) as tc:
        with (
            tc.tile_pool(name="const", bufs=1) as cpool,
            tc.tile_pool(name="htpool", bufs=1) as htpool,
            tc.tile_pool(name="gath", bufs=2) as gpool,
            tc.tile_pool(name="kagg", bufs=2) as kpool,
            tc.tile_pool(name="work", bufs=2) as wpool,
            tc.tile_pool(name="big", bufs=1) as xpool,
            tc.tile_pool(name="ps_e", bufs=2, space="PSUM") as pse,
            tc.tile_pool(name="ps_h", bufs=2, space="PSUM") as psh,
            tc.tile_pool(name="ps_m", bufs=2, space="PSUM") as psm,
            tc.tile_pool(name="ps_y", bufs=1, space="PSUM") as psy,
        ):
            # ---- constants ----
            aw1_sb = cpool.tile([D, H_ATT], BF16, tag="aw1")
            nc.sync.dma_start(aw1_sb[:], aw1[:])
            aw2r_sb = cpool.tile([128, 1], BF16, tag="aw2r")
            nc.sync.dma_start(aw2r_sb[:], aw2r[:])
            ab1s_sb = cpool.tile([128, 1], F32, tag="ab1s")
            nc.sync.dma_start(ab1s_sb[:], ab1s[:])
            ab2s_sb = cpool.tile([128, 1], F32, tag="ab2s")
            nc.sync.dma_start(ab2s_sb[:], ab2s[:])
            hw1_sb = cpool.tile([D, D], BF16, tag="hw1")
            nc.sync.dma_start(hw1_sb[:], hw1[:])
            hw2_sb = cpool.tile([D, D], BF16, tag="hw2")
            nc.sync.dma_start(hw2_sb[:], hw2[:])
            idbf_sb = cpool.tile([128, 128], BF16, tag="idbf")
            nc.sync.dma_start(idbf_sb[:], id_bf[:])
            idf_sb = cpool.tile([128, 128], F32, tag="idf")
            nc.sync.dma_start(idf_sb[:], id_f32[:])
            ind4_sb = cpool.tile([1, 512], F32, tag="ind4")
            nc.sync.dma_start(ind4_sb[:], ind4[:])
            ones_sb = cpool.tile([128, 1], BF16, tag="ones")
            nc.sync.dma_start(ones_sb[:], ones_col[:])
            mdiag_sb = cpool.tile([128, NT * 4], BF16, tag="mdiag")
            nc.sync.dma_start(mdiag_sb[:], mdiag[:])
            gtabt_sb = cpool.tile([D, GPC], F32, tag="gtabt")
            nc.sync.dma_start(gtabt_sb[:], gtabt[:])
            if gather == "device":
                gidx_sb = cpool.tile([128, NT], I32, tag="gidx")
                nc.sync.dma_start(gidx_sb[:], gidx[:])

            # ---- H^T resident [128, KT*GPC] bf16 (16MB) ----
            ht_sb = htpool.tile([128, KT * GPC], BF16, tag="ht")
            for k in range(KT):
                nc.sync.dma_start(
                    ht_sb[:, k * GPC : (k + 1) * GPC],
                    ht[k * 128 : (k + 1) * 128, :],
                )

            # ---- attention over 8 superblocks ----
            def fill_gbig(sbi, tile_):
                if gather == "device":
                    for k in range(32):
                        t = sbi * 32 + k
                        nc.gpsimd.indirect_dma_start(
                            out=tile_[:, k * 128 : (k + 1) * 128],
                            out_offset=None,
                            in_=sum_tab[:],
                            in_offset=bass.IndirectOffsetOnAxis(
                                ap=gidx_sb[:, t : t + 1], axis=0
                            ),
                        )
                else:
                    nc.scalar.dma_start(
                        tile_[:], memb[:, sbi * 32 * 128 : (sbi + 1) * 32 * 128]
                    )

            vag_sb = [None, None]
            gbig = gpool.tile([128, 32 * 128], BF16, tag="gbig", name="gbig0")
            fill_gbig(0, gbig)
            etbig = None
            for sb in range(NSB):
                if sb + 1 < NSB:
                    gbig_nxt = gpool.tile(
                        [128, 32 * 128], BF16, tag="gbig", name=f"gbig{sb + 1}"
                    )
                    fill_gbig(sb + 1, gbig_nxt)

                misc = psm.tile([128, 512], F32, tag="misc")
                logit_ps = misc[:, 0:32]
                den_ps = misc[:, 32:33]
                denbc_ps = misc[:, 36:68]
                dent_ps = misc[0:1, 68:196]
                gatt_ps = misc[:, 196:324]
                v_ps = misc[:, 324:452]

                # chunks of 512 rows; hid^T for 3 chunks stacked per PSUM
                # tile at partition offsets 0/32/64 (96 is not encodable)
                embT_sb = None
                hid_ps = None
                for c in range(8):
                    if c % 2 == 0:
                        embT_ps = pse.tile([128, 1024], BF16, tag="embT")
                        for l in range(2):
                            for s in range(4):
                                t = 4 * (c + l) + s
                                nc.tensor.transpose(
                                    embT_ps[:, l * 512 + s * 128 : l * 512 + (s + 1) * 128],
                                    gbig[:, t * 128 : (t + 1) * 128],
                                    idbf_sb[:],
                                )
                        embT_sb = wpool.tile([128, 1024], BF16, tag="embT_sb")
                        nc.vector.tensor_copy(embT_sb[:], embT_ps[:])
                    embT_src = embT_sb[:, (c % 2) * 512 : (c % 2 + 1) * 512]
                    q, j = c // 3, c % 3
                    if j == 0:
                        hid_ps = psh.tile([128, 512], F32, tag="hid")
                        hid_tiles = hid_ps
                    nc.tensor.matmul(
                        hid_tiles[32 * j : 32 * j + 16, :],
                        aw1_sb[:],
                        embT_src,
                        start=True,
                        stop=True,
                    )
                    if c in (2, 5, 7):
                        nj = 3 if c != 7 else 2
                        hidT_sb = wpool.tile([128, 512], BF16, tag="hidT")
                        nc.scalar.activation(
                            hidT_sb[:], hid_tiles[:], AF.Relu, bias=ab1s_sb[:, :1]
                        )
                        for jj in range(nj):
                            for s in range(4):
                                tl = 4 * (3 * q + jj) + s  # tile in superblock
                                nc.tensor.matmul(
                                    logit_ps[:, tl : tl + 1],
                                    hidT_sb[32 * jj : 32 * jj + 16, s * 128 : (s + 1) * 128],
                                    aw2r_sb[32 * jj : 32 * jj + 16, :],
                                    start=True,
                                    stop=True,
                                )

                p_sb = wpool.tile([128, 32], BF16, tag="p")
                nc.scalar.activation(p_sb[:], logit_ps, AF.Exp, bias=ab2s_sb[:, :1])
                pm_sb = wpool.tile([128, 128], BF16, tag="pm")
                nc.vector.tensor_tensor(
                    pm_sb[:].rearrange("p (t l) -> p t l", l=4),
                    p_sb[:].rearrange("p (t o) -> p t o", o=1).to_broadcast(
                        [128, 32, 4]
                    ),
                    mdiag_sb[:, sb * 128 : (sb + 1) * 128].rearrange(
                        "p (t l) -> p t l", l=4
                    ),
                    mybir.AluOpType.mult,
                )
                # denominators: den[col] = sum_rows pm[row, col]
                nc.tensor.matmul(den_ps, pm_sb[:], ones_sb[:], start=True, stop=True)
                den_sb = wpool.tile([128, 1], F32, tag="den")
                nc.vector.tensor_copy(den_sb[:], den_ps)
                nc.tensor.transpose(dent_ps, den_sb[:], idf_sb[:])
                dent_sb = wpool.tile([1, 128], F32, tag="dent")
                nc.vector.tensor_copy(dent_sb[:], dent_ps)
                # den_bc[r, t] = den[4t + r//32] via 4 K=1 matmuls
                dent_re = dent_sb[:].rearrange("p (t l) -> p t l", l=4)
                for gl in range(4):
                    nc.tensor.matmul(
                        denbc_ps,
                        ind4_sb[0:1, gl * 128 : (gl + 1) * 128],
                        dent_re[:, :, gl : gl + 1],
                        start=(gl == 0),
                        stop=(gl == 3),
                    )
                recip_sb = wpool.tile([128, 32], F32, tag="recip")
                nc.vector.reciprocal(recip_sb[:], denbc_ps)
                maskp_sb = wpool.tile([128, 128], BF16, tag="maskp")
                nc.vector.tensor_tensor(
                    maskp_sb[:].rearrange("p (t l) -> p t l", l=4),
                    recip_sb[:].rearrange("p (t o) -> p t o", o=1).to_broadcast(
                        [128, 32, 4]
                    ),
                    pm_sb[:].rearrange("p (t l) -> p t l", l=4),
                    mybir.AluOpType.mult,
                )
                # g_att^T accumulation: [128 d, 128 groups]
                for t in range(32):
                    nc.tensor.matmul(
                        gatt_ps[:, 4 * t : 4 * t + 4],
                        gbig[:, t * 128 : (t + 1) * 128],
                        maskp_sb[:, 4 * t : 4 * t + 4],
                        start=True,
                        stop=True,
                    )
                xt_sb = wpool.tile([128, 128], BF16, tag="xt")
                nc.vector.tensor_tensor(
                    xt_sb[:],
                    gatt_ps,
                    gtabt_sb[:, sb * 128 : (sb + 1) * 128],
                    mybir.AluOpType.add,
                )
                nc.tensor.matmul(v_ps, xt_sb[:], hw1_sb[:], start=True, stop=True)
                v_sb = wpool.tile([128, 128], BF16, tag="v")
                nc.vector.tensor_copy(v_sb[:], v_ps)
                nc.scalar.dma_start(
                    vd[sb // 4][:, (sb % 4) * 128 : (sb % 4 + 1) * 128], v_sb[:]
                )
                if sb in (3, 7):
                    h = sb // 4
                    allgather(vd[h], vag[h])
                    vag_sb[h] = kpool.tile(
                        [128, NC * HGPC], BF16, tag="kh", name=f"vag_sb{h}"
                    )
                    nc.sync.dma_start(
                        vag_sb[h][:].rearrange("p (c f) -> p c f", f=HGPC),
                        vag[h].rearrange("(c p) f -> p c f", p=128),
                    )
                if sb + 1 < NSB:
                    gbig = gbig_nxt

            # ---- stage 1: Y^T = V^T H^T (k-split across AG halves) ----
            y_ps = [
                psy.tile([128, 512], F32, tag="y0", name="y_ps0"),
                psy.tile([128, 512], F32, tag="y1", name="y_ps1"),
            ]
            for h in range(2):
                for c2 in range(2):
                    for kk in range(32):
                        k = h * 32 + kk
                        nc.tensor.matmul(
                            y_ps[c2][:],
                            vag_sb[h][:, kk * 128 : (kk + 1) * 128],
                            ht_sb[:, k * GPC + c2 * 512 : k * GPC + c2 * 512 + 512],
                            start=(k == 0),
                            stop=(k == KT - 1),
                        )
            ht_all = xpool.tile([128, GPC], BF16, tag="hT")
            wag_sb = [None, None]
            for c2 in range(2):
                nc.scalar.activation(
                    ht_all[:, c2 * 512 : (c2 + 1) * 512], y_ps[c2][:], AF.Relu
                )
                # W = h @ hw2 for this half's groups + its AllGather
                for gb in range(4):
                    g0 = c2 * 4 + gb
                    wmisc = psm.tile([128, 512], F32, tag="misc")
                    w_ps = wmisc[:, 0:128]
                    nc.tensor.matmul(
                        w_ps,
                        ht_all[:, g0 * 128 : (g0 + 1) * 128],
                        hw2_sb[:],
                        start=True,
                        stop=True,
                    )
                    w_sb = wpool.tile([128, 128], BF16, tag="w")
                    nc.vector.tensor_copy(w_sb[:], w_ps)
                    nc.scalar.dma_start(
                        wd[c2][:, gb * 128 : (gb + 1) * 128], w_sb[:]
                    )
                allgather(wd[c2], wag[c2])
                wag_sb[c2] = kpool.tile(
                    [128, NC * HGPC], BF16, tag="kh", name=f"wag_sb{c2}"
                )
                nc.sync.dma_start(
                    wag_sb[c2][:].rearrange("p (c f) -> p c f", f=HGPC),
                    wag[c2].rearrange("(c p) f -> p c f", p=128),
                )

            # ---- stage 2: out^T = W^T H^T ----
            o_ps = [
                psy.tile([128, 512], F32, tag="y0", name="o_ps0"),
                psy.tile([128, 512], F32, tag="y1", name="o_ps1"),
            ]
            for h in range(2):
                for c2 in range(2):
                    for kk in range(32):
                        k = h * 32 + kk
                        nc.tensor.matmul(
                            o_ps[c2][:],
                            wag_sb[h][:, kk * 128 : (kk + 1) * 128],
                            ht_sb[:, k * GPC + c2 * 512 : k * GPC + c2 * 512 + 512],
                            start=(k == 0),
                            stop=(k == KT - 1),
                        )
            for c2 in range(2):
                ot_sb = xpool.tile([128, 512], F32, tag=f"ot{c2}")
                nc.vector.tensor_copy(ot_sb[:], o_ps[c2][:])
                nc.sync.dma_start(outt[:, c2 * 512 : (c2 + 1) * 512], ot_sb[:])

    nc.compile()
    return nc


def _prep_inputs(group_inputs, members, member_mask, user_embedding, H_gl,
                 user_table, group_table, aw1, ab1, aw2, ab2, hw1, hw2,
                 gather="device"):
    bf = ml_dtypes.bfloat16
    sum_tab = (
        np.asarray(user_table, np.float32) + np.asarray(user_embedding, np.float32)
    )
    gi = np.asarray(group_inputs, np.int64)
    gtab_full = np.asarray(group_table, np.float32)[gi]
    Hg = np.asarray(H_gl, np.float32)

    aw2v = np.asarray(aw2, np.float32).reshape(-1)
    ab1v = np.asarray(ab1, np.float32).reshape(-1)
    aw2r = np.zeros((128, 1), np.float32)
    ab1s = np.zeros((128, 1), np.float32)
    for j in range(4):
        aw2r[32 * j : 32 * j + H_ATT, 0] = aw2v
        ab1s[32 * j : 32 * j + H_ATT, 0] = ab1v
    ab2s = np.full((128, 1), np.asarray(ab2, np.float32).reshape(-1)[0], np.float32)
    ind4 = np.zeros((1, 512), np.float32)
    for gl in range(4):
        ind4[0, gl * 128 + 32 * gl : gl * 128 + 32 * (gl + 1)] = 1.0

    # ht row-permutation matching the transposed-tiled AllGather layout:
    # k-tile (h, c, j) holds global groups c*GPC + h*HGPC + j*128 + p
    perm = np.concatenate(
        [
            np.arange(c * GPC + h * HGPC + j * 128, c * GPC + h * HGPC + (j + 1) * 128)
            for h in range(2)
            for c in range(NC)
            for j in range(4)
        ]
    )

    consts = dict(
        aw1=np.asarray(aw1, np.float32).astype(bf),
        aw2r=aw2r.astype(bf),
        ab1s=ab1s,
        ab2s=ab2s,
        hw1=np.asarray(hw1, np.float32).astype(bf),
        hw2=np.asarray(hw2, np.float32).astype(bf),
        id_bf=np.eye(128, dtype=np.float32).astype(bf),
        id_f32=np.eye(128, dtype=np.float32),
        ind4=ind4,
        ones_col=np.ones((128, 1), np.float32).astype(bf),
    )
    if gather == "device":
        consts["sum_tab"] = sum_tab.astype(bf)

    p = np.arange(128)
    gl_p = p // 32
    m_p = p % 32
    t_idx = np.arange(NT)
    in_maps = []
    for c in range(NC):
        sl = slice(c * GPC, (c + 1) * GPC)
        mem = np.asarray(members, np.int64)[sl].astype(np.int32).reshape(-1)
        mask01 = (np.asarray(member_mask, np.float32)[sl] > 0).astype(np.float32)
        val = mask01[(4 * t_idx[None, :] + gl_p[:, None]), m_p[:, None]]  # [128, NT]
        mdiag = np.zeros((128, NT, 4), np.float32)
        mdiag[p, :, gl_p] = val
        m = dict(
            consts,
            mdiag=np.ascontiguousarray(mdiag.reshape(128, NT * 4)).astype(bf),
            gtabt=np.ascontiguousarray(gtab_full[sl].T),
            ht=np.ascontiguousarray(Hg[sl].T[perm]).astype(bf),
        )
        if gather == "device":
            m["gidx"] = np.ascontiguousarray(mem.reshape(NT, 128).T)
        else:
            mb = sum_tab[mem.reshape(NT, 128)].astype(bf)  # [NT, 128, D]
            m["memb"] = np.ascontiguousarray(
                mb.transpose(1, 0, 2)
            ).reshape(128, NT * D)
        in_maps.append(m)
    return in_maps


def kernel(**inputs):
    gather = os.environ.get("KGATHER", "device")
    key = f"nc_{gather}"
    if key not in _CACHE:
        _CACHE[key] = _build(gather)
        _CACHE["nc"] = _CACHE[key]
    nc = _CACHE[key]
    in_maps = _prep_inputs(**inputs, gather=gather)
    res = run_bass_kernel_spmd(nc, in_maps, core_ids=list(range(NC)))
    out = np.concatenate(
        [np.ascontiguousarray(res.results[c]["outt"].T) for c in range(NC)], axis=0
    )
    return out.astype(np.float32)


if __name__ == "__main__":
    import reference
    inp = {k: np.asarray(v) for k, v in reference.setup_inputs().items()}
    exp = np.asarray(reference.reference(**inp))
    got = kernel(**inp)
    err = np.abs(got - exp).max() / (np.abs(exp).max() + 1e-30)
    rel = np.linalg.norm(got - exp) / (np.linalg.norm(exp) + 1e-30)
    print(f"absmax-rel: {err:.3e}  fro-rel: {rel:.3e}")


# revision 49
# speedup vs baseline: 1.5228x; 1.5228x over previous
"""Trainium2 Bass kernel for nn_HHGR (gnn_message_passing).

Strategy (8 NeuronCores, groups sharded 1024/core):
  host prep: sum_tab = user_table + user_embedding (bf16), per-core shards
    of members/mask, H^T slabs (bf16, k-rows permuted to AllGather order),
    gtab^T, replicated small weights.
  device per core (1024 groups = 8 superblocks x 128 groups = 256 tiles):
    * gather member rows (bf16) - either 32 indirect DMAs per superblock
      (KGATHER=device) or one sequential DMA of host-pregathered slab
      (KGATHER=host)
    * attention, per superblock:
        PE-transpose emb tiles -> embT chunks; hid^T = aw1^T @ embT with
        4 chunks stacked in one PSUM bank (partition offsets 0/32/64/96);
        one batched Relu+bias; per-tile natural logits (lhsT = hidT slice,
        rhs = aw2); one batched Exp+bias -> p [128,32]; pm = p * mask;
        denominators via one matmul; 1/den broadcast to rows via 4 tiny
        K=1 matmuls; maskp = pm * recip; g_att^T accumulated per tile
        (lhsT = emb tile, rhs = maskp cols); X^T = g_att^T + gtab^T;
        V = X @ hw1 -> vdram half.
    * AllGather V in two halves (issued after superblock 3 and 7) so the
      first collective hides under attention of superblocks 4-7.
    * stage 1: Y^T = V^T H^T, k-tiles of half 1 accumulate while half 2's
      AllGather is in flight; relu -> h^T; W = h @ hw2 per column half,
      each followed by its W-AllGather; stage 2 (out^T = W^T H^T) k-tiles
      of half 1 overlap the second W-AllGather.
    * out^T [128, 1024] f32 written once; host transposes.
"""
import sys
sys.path.insert(0, "/opt/trn_rl_repo")

import os
import numpy as np
import ml_dtypes

import concourse.bass as bass
import concourse.bacc as bacc
import concourse.mybir as mybir
import concourse.tile as tile
from concourse.bass_utils import run_bass_kernel_spmd

F32 = mybir.dt.float32
BF16 = mybir.dt.bfloat16
I32 = mybir.dt.int32
AF = mybir.ActivationFunctionType

G, M, D, U = 8192, 32, 128, 200000
H_ATT = 16
NC = 8
GPC = G // NC          # 1024 groups per core
R = GPC * M            # 32768 member rows per core
NT = R // 128          # 256 tiles of 128 rows
NSB = GPC // 128       # 8 superblocks of 128 groups (32 tiles each)
KT = G // 128          # 64 k-tiles for the big matmuls
HGPC = GPC // 2        # half of the per-core groups (AllGather halves)

_CACHE = {}


def _build(gather="device"):
    nc = bacc.Bacc("TRN2", target_bir_lowering=False, debug=False)

    # ---- inputs ----
    if gather == "device":
        sum_tab = nc.dram_tensor("sum_tab", [U, D], BF16, kind="ExternalInput")
        gidx = nc.dram_tensor("gidx", [128, NT], I32, kind="ExternalInput")
    else:
        memb = nc.dram_tensor("memb", [128, NT * D], BF16, kind="ExternalInput")
    mdiag = nc.dram_tensor("mdiag", [128, NT * 4], BF16, kind="ExternalInput")
    gtabt = nc.dram_tensor("gtabt", [D, GPC], F32, kind="ExternalInput")
    ht = nc.dram_tensor("ht", [G, GPC], BF16, kind="ExternalInput")
    aw1 = nc.dram_tensor("aw1", [D, H_ATT], BF16, kind="ExternalInput")
    aw2r = nc.dram_tensor("aw2r", [128, 1], BF16, kind="ExternalInput")
    ab1s = nc.dram_tensor("ab1s", [128, 1], F32, kind="ExternalInput")
    ab2s = nc.dram_tensor("ab2s", [128, 1], F32, kind="ExternalInput")
    hw1 = nc.dram_tensor("hw1", [D, D], BF16, kind="ExternalInput")
    hw2 = nc.dram_tensor("hw2", [D, D], BF16, kind="ExternalInput")
    id_bf = nc.dram_tensor("id_bf", [128, 128], BF16, kind="ExternalInput")
    id_f32 = nc.dram_tensor("id_f32", [128, 128], F32, kind="ExternalInput")
    ind4 = nc.dram_tensor("ind4", [1, 512], F32, kind="ExternalInput")
    ones_col = nc.dram_tensor("ones_col", [128, 1], BF16, kind="ExternalInput")

    outt = nc.dram_tensor("outt", [D, GPC], F32, kind="ExternalOutput")

    # internal DRAM for collectives — per-core quarters, stored transposed-
    # tiled [128, 256] so the post-AllGather reload is 1KB-contiguous per
    # partition: vd[q][p, j*128+d] = V[local group q*256 + j*128 + p, d]
    QG = GPC // 4
    vd = [nc.dram_tensor(f"vd{q}", [128, QG], BF16) for q in range(4)]
    vag = [
        nc.dram_tensor(f"vag{q}", [NC * 128, QG], BF16, addr_space="Shared")
        for q in range(4)
    ]
    wd = [nc.dram_tensor(f"wd{q}", [128, QG], BF16) for q in range(4)]
    wag = [
        nc.dram_tensor(f"wag{q}", [NC * 128, QG], BF16, addr_space="Shared")
        for q in range(4)
    ]

    def allgather(src, dst):
        nc.gpsimd.collective_compute(
            "AllGather",
            mybir.AluOpType.bypass,
            replica_groups=[list(range(NC))],
            ins=[src[:]],
            outs=[dst[:]],
        )

    with tile.TileContext(nc) as tc:
        with (
            tc.tile_pool(name="const", bufs=1) as cpool,
            tc.tile_pool(name="htpool", bufs=1) as htpool,
            tc.tile_pool(name="gath", bufs=2) as gpool,
            tc.tile_pool(name="kagg", bufs=4) as kpool,
            tc.tile_pool(name="work", bufs=2) as wpool,
            tc.tile_pool(name="big", bufs=1) as xpool,
            tc.tile_pool(name="ps_e", bufs=2, space="PSUM") as pse,
            tc.tile_pool(name="ps_h", bufs=2, space="PSUM") as psh,
            tc.tile_pool(name="ps_m", bufs=2, space="PSUM") as psm,
            tc.tile_pool(name="ps_y", bufs=1, space="PSUM") as psy,
        ):
            # ---- constants ----
            aw1_sb = cpool.tile([D, H_ATT], BF16, tag="aw1")
            nc.sync.dma_start(aw1_sb[:], aw1[:])
            aw2r_sb = cpool.tile([128, 1], BF16, tag="aw2r")
            nc.sync.dma_start(aw2r_sb[:], aw2r[:])
            ab1s_sb = cpool.tile([128, 1], F32, tag="ab1s")
            nc.sync.dma_start(ab1s_sb[:], ab1s[:])
            ab2s_sb = cpool.tile([128, 1], F32, tag="ab2s")
            nc.sync.dma_start(ab2s_sb[:], ab2s[:])
            hw1_sb = cpool.tile([D, D], BF16, tag="hw1")
            nc.sync.dma_start(hw1_sb[:], hw1[:])
            hw2_sb = cpool.tile([D, D], BF16, tag="hw2")
            nc.sync.dma_start(hw2_sb[:], hw2[:])
            idbf_sb = cpool.tile([128, 128], BF16, tag="idbf")
            nc.sync.dma_start(idbf_sb[:], id_bf[:])
            idf_sb = cpool.tile([128, 128], F32, tag="idf")
            nc.sync.dma_start(idf_sb[:], id_f32[:])
            ind4_sb = cpool.tile([1, 512], F32, tag="ind4")
            nc.sync.dma_start(ind4_sb[:], ind4[:])
            ones_sb = cpool.tile([128, 1], BF16, tag="ones")
            nc.sync.dma_start(ones_sb[:], ones_col[:])
            mdiag_sb = cpool.tile([128, NT * 4], BF16, tag="mdiag")
            nc.sync.dma_start(mdiag_sb[:], mdiag[:])
            gtabt_sb = cpool.tile([D, GPC], F32, tag="gtabt")
            nc.sync.dma_start(gtabt_sb[:], gtabt[:])
            if gather == "device":
                gidx_sb = cpool.tile([128, NT], I32, tag="gidx")
                nc.sync.dma_start(gidx_sb[:], gidx[:])

            # ---- H^T resident [128, KT*GPC] bf16 (16MB) ----
            ht_sb = htpool.tile([128, KT * GPC], BF16, tag="ht")
            for k in range(KT):
                nc.sync.dma_start(
                    ht_sb[:, k * GPC : (k + 1) * GPC],
                    ht[k * 128 : (k + 1) * 128, :],
                )

            # ---- attention over 8 superblocks ----
            def fill_gbig(sbi, tile_):
                if gather == "device":
                    for k in range(32):
                        t = sbi * 32 + k
                        nc.gpsimd.indirect_dma_start(
                            out=tile_[:, k * 128 : (k + 1) * 128],
                            out_offset=None,
                            in_=sum_tab[:],
                            in_offset=bass.IndirectOffsetOnAxis(
                                ap=gidx_sb[:, t : t + 1], axis=0
                            ),
                        )
                else:
                    nc.scalar.dma_start(
                        tile_[:], memb[:, sbi * 32 * 128 : (sbi + 1) * 32 * 128]
                    )

            vag_sb = [None, None, None, None]
            gbig = gpool.tile([128, 32 * 128], BF16, tag="gbig", name="gbig0")
            fill_gbig(0, gbig)
            etbig = None
            for sb in range(NSB):
                if sb + 1 < NSB:
                    gbig_nxt = gpool.tile(
                        [128, 32 * 128], BF16, tag="gbig", name=f"gbig{sb + 1}"
                    )
                    fill_gbig(sb + 1, gbig_nxt)

                misc = psm.tile([128, 512], F32, tag="misc")
                logit_ps = misc[:, 0:32]
                den_ps = misc[:, 32:33]
                denbc_ps = misc[:, 36:68]
                dent_ps = misc[0:1, 68:196]
                gatt_ps = misc[:, 196:324]
                v_ps = misc[:, 324:452]

                # chunks of 512 rows; hid^T for 3 chunks stacked per PSUM
                # tile at partition offsets 0/32/64 (96 is not encodable)
                embT_sb = None
                hid_ps = None
                for c in range(8):
                    if c % 2 == 0:
                        embT_ps = pse.tile([128, 1024], BF16, tag="embT")
                        for l in range(2):
                            for s in range(4):
                                t = 4 * (c + l) + s
                                nc.tensor.transpose(
                                    embT_ps[:, l * 512 + s * 128 : l * 512 + (s + 1) * 128],
                                    gbig[:, t * 128 : (t + 1) * 128],
                                    idbf_sb[:],
                                )
                        embT_sb = wpool.tile([128, 1024], BF16, tag="embT_sb")
                        nc.vector.tensor_copy(embT_sb[:], embT_ps[:])
                    embT_src = embT_sb[:, (c % 2) * 512 : (c % 2 + 1) * 512]
                    q, j = c // 3, c % 3
                    if j == 0:
                        hid_ps = psh.tile([128, 512], F32, tag="hid")
                        hid_tiles = hid_ps
                    nc.tensor.matmul(
                        hid_tiles[32 * j : 32 * j + 16, :],
                        aw1_sb[:],
                        embT_src,
                        start=True,
                        stop=True,
                    )
                    if c in (2, 5, 7):
                        nj = 3 if c != 7 else 2
                        hidT_sb = wpool.tile([128, 512], BF16, tag="hidT")
                        nc.scalar.activation(
                            hidT_sb[:], hid_tiles[:], AF.Relu, bias=ab1s_sb[:, :1]
                        )
                        for jj in range(nj):
                            for s in range(4):
                                tl = 4 * (3 * q + jj) + s  # tile in superblock
                                nc.tensor.matmul(
                                    logit_ps[:, tl : tl + 1],
                                    hidT_sb[32 * jj : 32 * jj + 16, s * 128 : (s + 1) * 128],
                                    aw2r_sb[32 * jj : 32 * jj + 16, :],
                                    start=True,
                                    stop=True,
                                )

                p_sb = wpool.tile([128, 32], BF16, tag="p")
                nc.scalar.activation(p_sb[:], logit_ps, AF.Exp, bias=ab2s_sb[:, :1])
                pm_sb = wpool.tile([128, 128], BF16, tag="pm")
                nc.vector.tensor_tensor(
                    pm_sb[:].rearrange("p (t l) -> p t l", l=4),
                    p_sb[:].rearrange("p (t o) -> p t o", o=1).to_broadcast(
                        [128, 32, 4]
                    ),
                    mdiag_sb[:, sb * 128 : (sb + 1) * 128].rearrange(
                        "p (t l) -> p t l", l=4
                    ),
                    mybir.AluOpType.mult,
                )
                # denominators: den[col] = sum_rows pm[row, col]
                nc.tensor.matmul(den_ps, pm_sb[:], ones_sb[:], start=True, stop=True)
                den_sb = wpool.tile([128, 1], F32, tag="den")
                nc.vector.tensor_copy(den_sb[:], den_ps)
                nc.tensor.transpose(dent_ps, den_sb[:], idf_sb[:])
                dent_sb = wpool.tile([1, 128], F32, tag="dent")
                nc.vector.tensor_copy(dent_sb[:], dent_ps)
                # den_bc[r, t] = den[4t + r//32] via 4 K=1 matmuls
                dent_re = dent_sb[:].rearrange("p (t l) -> p t l", l=4)
                for gl in range(4):
                    nc.tensor.matmul(
                        denbc_ps,
                        ind4_sb[0:1, gl * 128 : (gl + 1) * 128],
                        dent_re[:, :, gl : gl + 1],
                        start=(gl == 0),
                        stop=(gl == 3),
                    )
                recip_sb = wpool.tile([128, 32], F32, tag="recip")
                nc.vector.reciprocal(recip_sb[:], denbc_ps)
                maskp_sb = wpool.tile([128, 128], BF16, tag="maskp")
                nc.vector.tensor_tensor(
                    maskp_sb[:].rearrange("p (t l) -> p t l", l=4),
                    recip_sb[:].rearrange("p (t o) -> p t o", o=1).to_broadcast(
                        [128, 32, 4]
                    ),
                    pm_sb[:].rearrange("p (t l) -> p t l", l=4),
                    mybir.AluOpType.mult,
                )
                # g_att^T accumulation: [128 d, 128 groups]
                for t in range(32):
                    nc.tensor.matmul(
                        gatt_ps[:, 4 * t : 4 * t + 4],
                        gbig[:, t * 128 : (t + 1) * 128],
                        maskp_sb[:, 4 * t : 4 * t + 4],
                        start=True,
                        stop=True,
                    )
                xt_sb = wpool.tile([128, 128], BF16, tag="xt")
                nc.vector.tensor_tensor(
                    xt_sb[:],
                    gatt_ps,
                    gtabt_sb[:, sb * 128 : (sb + 1) * 128],
                    mybir.AluOpType.add,
                )
                nc.tensor.matmul(v_ps, xt_sb[:], hw1_sb[:], start=True, stop=True)
                v_sb = wpool.tile([128, 128], BF16, tag="v")
                nc.vector.tensor_copy(v_sb[:], v_ps)
                nc.scalar.dma_start(
                    vd[sb // 2][:, (sb % 2) * 128 : (sb % 2 + 1) * 128], v_sb[:]
                )
                if sb % 2 == 1:
                    qq = sb // 2
                    allgather(vd[qq], vag[qq])
                    vag_sb[qq] = kpool.tile(
                        [128, NC * 256], BF16, tag="kq", name=f"vag_sb{qq}"
                    )
                    nc.sync.dma_start(
                        vag_sb[qq][:].rearrange("p (c f) -> p c f", f=256),
                        vag[qq].rearrange("(c p) f -> p c f", p=128),
                    )
                if sb + 1 < NSB:
                    gbig = gbig_nxt

            # ---- stage 1: Y^T = V^T H^T (k-split across AG halves) ----
            y_ps = [
                psy.tile([128, 512], F32, tag="y0", name="y_ps0"),
                psy.tile([128, 512], F32, tag="y1", name="y_ps1"),
            ]
            for q in range(4):
                for c2 in range(2):
                    for kk in range(16):
                        k = q * 16 + kk
                        nc.tensor.matmul(
                            y_ps[c2][:],
                            vag_sb[q][:, kk * 128 : (kk + 1) * 128],
                            ht_sb[:, k * GPC + c2 * 512 : k * GPC + c2 * 512 + 512],
                            start=(k == 0),
                            stop=(k == KT - 1),
                        )
            ht_all = xpool.tile([128, GPC], BF16, tag="hT")
            wag_sb = [None, None, None, None]
            for c2 in range(2):
                nc.scalar.activation(
                    ht_all[:, c2 * 512 : (c2 + 1) * 512], y_ps[c2][:], AF.Relu
                )
                # W = h @ hw2 per quarter of this half's groups + AllGather
                for wq in range(2 * c2, 2 * c2 + 2):
                    for gb in range(2):
                        g0 = wq * 2 + gb
                        wmisc = psm.tile([128, 512], F32, tag="misc")
                        w_ps = wmisc[:, 0:128]
                        nc.tensor.matmul(
                            w_ps,
                            ht_all[:, g0 * 128 : (g0 + 1) * 128],
                            hw2_sb[:],
                            start=True,
                            stop=True,
                        )
                        w_sb = wpool.tile([128, 128], BF16, tag="w")
                        nc.vector.tensor_copy(w_sb[:], w_ps)
                        nc.scalar.dma_start(
                            wd[wq][:, gb * 128 : (gb + 1) * 128], w_sb[:]
                        )
                    allgather(wd[wq], wag[wq])
                    wag_sb[wq] = kpool.tile(
                        [128, NC * 256], BF16, tag="kq", name=f"wag_sb{wq}"
                    )
                    nc.sync.dma_start(
                        wag_sb[wq][:].rearrange("p (c f) -> p c f", f=256),
                        wag[wq].rearrange("(c p) f -> p c f", p=128),
                    )

            # ---- stage 2: out^T = W^T H^T ----
            o_ps = [
                psy.tile([128, 512], F32, tag="y0", name="o_ps0"),
                psy.tile([128, 512], F32, tag="y1", name="o_ps1"),
            ]
            for q in range(4):
                for c2 in range(2):
                    for kk in range(16):
                        k = q * 16 + kk
                        nc.tensor.matmul(
                            o_ps[c2][:],
                            wag_sb[q][:, kk * 128 : (kk + 1) * 128],
                            ht_sb[:, k * GPC + c2 * 512 : k * GPC + c2 * 512 + 512],
                            start=(k == 0),
                            stop=(k == KT - 1),
                        )
            for c2 in range(2):
                ot_sb = xpool.tile([128, 512], F32, tag=f"ot{c2}")
                nc.vector.tensor_copy(ot_sb[:], o_ps[c2][:])
                nc.sync.dma_start(outt[:, c2 * 512 : (c2 + 1) * 512], ot_sb[:])

    nc.compile()
    return nc


def _prep_inputs(group_inputs, members, member_mask, user_embedding, H_gl,
                 user_table, group_table, aw1, ab1, aw2, ab2, hw1, hw2,
                 gather="device"):
    bf = ml_dtypes.bfloat16
    sum_tab = (
        np.asarray(user_table, np.float32) + np.asarray(user_embedding, np.float32)
    )
    gi = np.asarray(group_inputs, np.int64)
    gtab_full = np.asarray(group_table, np.float32)[gi]
    Hg = np.asarray(H_gl, np.float32)

    aw2v = np.asarray(aw2, np.float32).reshape(-1)
    ab1v = np.asarray(ab1, np.float32).reshape(-1)
    aw2r = np.zeros((128, 1), np.float32)
    ab1s = np.zeros((128, 1), np.float32)
    for j in range(4):
        aw2r[32 * j : 32 * j + H_ATT, 0] = aw2v
        ab1s[32 * j : 32 * j + H_ATT, 0] = ab1v
    ab2s = np.full((128, 1), np.asarray(ab2, np.float32).reshape(-1)[0], np.float32)
    ind4 = np.zeros((1, 512), np.float32)
    for gl in range(4):
        ind4[0, gl * 128 + 32 * gl : gl * 128 + 32 * (gl + 1)] = 1.0

    # ht row-permutation matching the transposed-tiled AllGather layout:
    # k-tile (q, c, j) holds global groups c*GPC + q*256 + j*128 + p
    perm = np.concatenate(
        [
            np.arange(c * GPC + q * 256 + j * 128, c * GPC + q * 256 + (j + 1) * 128)
            for q in range(4)
            for c in range(NC)
            for j in range(2)
        ]
    )

    consts = dict(
        aw1=np.asarray(aw1, np.float32).astype(bf),
        aw2r=aw2r.astype(bf),
        ab1s=ab1s,
        ab2s=ab2s,
        hw1=np.asarray(hw1, np.float32).astype(bf),
        hw2=np.asarray(hw2, np.float32).astype(bf),
        id_bf=np.eye(128, dtype=np.float32).astype(bf),
        id_f32=np.eye(128, dtype=np.float32),
        ind4=ind4,
        ones_col=np.ones((128, 1), np.float32).astype(bf),
    )
    if gather == "device":
        consts["sum_tab"] = sum_tab.astype(bf)

    p = np.arange(128)
    gl_p = p // 32
    m_p = p % 32
    t_idx = np.arange(NT)
    in_maps = []
    for c in range(NC):
        sl = slice(c * GPC, (c + 1) * GPC)
        mem = np.asarray(members, np.int64)[sl].astype(np.int32).reshape(-1)
        mask01 = (np.asarray(member_mask, np.float32)[sl] > 0).astype(np.float32)
        val = mask01[(4 * t_idx[None, :] + gl_p[:, None]), m_p[:, None]]  # [128, NT]
        mdiag = np.zeros((128, NT, 4), np.float32)
        mdiag[p, :, gl_p] = val
        m = dict(
            consts,
            mdiag=np.ascontiguousarray(mdiag.reshape(128, NT * 4)).astype(bf),
            gtabt=np.ascontiguousarray(gtab_full[sl].T),
            ht=np.ascontiguousarray(Hg[sl].T[perm]).astype(bf),
        )
        if gather == "device":
            m["gidx"] = np.ascontiguousarray(mem.reshape(NT, 128).T)
        else:
            mb = sum_tab[mem.reshape(NT, 128)].astype(bf)  # [NT, 128, D]
            m["memb"] = np.ascontiguousarray(
                mb.transpose(1, 0, 2)
            ).reshape(128, NT * D)
        in_maps.append(m)
    return in_maps


def kernel(**inputs):
    gather = os.environ.get("KGATHER", "device")
    key = f"nc_{gather}"
    if key not in _CACHE:
        _CACHE[key] = _build(gather)
        _CACHE["nc"] = _CACHE[key]
    nc = _CACHE[key]
    in_maps = _prep_inputs(**inputs, gather=gather)
    res = run_bass_kernel_spmd(nc, in_maps, core_ids=list(range(NC)))
    out = np.concatenate(
        [np.ascontiguousarray(res.results[c]["outt"].T) for c in range(NC)], axis=0
    )
    return out.astype(np.float32)


if __name__ == "__main__":
    import reference
    inp = {k: np.asarray(v) for k, v in reference.setup_inputs().items()}
    exp = np.asarray(reference.reference(**inp))
    got = kernel(**inp)
    err = np.abs(got - exp).max() / (np.abs(exp).max() + 1e-30)
    rel = np.linalg.norm(got - exp) / (np.linalg.norm(exp) + 1e-30)
    print(f"absmax-rel: {err:.3e}  fro-rel: {rel:.3e}")


# revision 50
# speedup vs baseline: 1.5270x; 1.0028x over previous
"""Trainium2 Bass kernel for nn_HHGR (gnn_message_passing).

Strategy (8 NeuronCores, groups sharded 1024/core):
  host prep: sum_tab = user_table + user_embedding (bf16), per-core shards
    of members/mask, H^T slabs (bf16, k-rows permuted to AllGather order),
    gtab^T, replicated small weights.
  device per core (1024 groups = 8 superblocks x 128 groups = 256 tiles):
    * gather member rows (bf16) - either 32 indirect DMAs per superblock
      (KGATHER=device) or one sequential DMA of host-pregathered slab
      (KGATHER=host)
    * attention, per superblock:
        PE-transpose emb tiles -> embT chunks; hid^T = aw1^T @ embT with
        4 chunks stacked in one PSUM bank (partition offsets 0/32/64/96);
        one batched Relu+bias; per-tile natural logits (lhsT = hidT slice,
        rhs = aw2); one batched Exp+bias -> p [128,32]; pm = p * mask;
        denominators via one matmul; 1/den broadcast to rows via 4 tiny
        K=1 matmuls; maskp = pm * recip; g_att^T accumulated per tile
        (lhsT = emb tile, rhs = maskp cols); X^T = g_att^T + gtab^T;
        V = X @ hw1 -> vdram half.
    * AllGather V in two halves (issued after superblock 3 and 7) so the
      first collective hides under attention of superblocks 4-7.
    * stage 1: Y^T = V^T H^T, k-tiles of half 1 accumulate while half 2's
      AllGather is in flight; relu -> h^T; W = h @ hw2 per column half,
      each followed by its W-AllGather; stage 2 (out^T = W^T H^T) k-tiles
      of half 1 overlap the second W-AllGather.
    * out^T [128, 1024] f32 written once; host transposes.
"""
import sys
sys.path.insert(0, "/opt/trn_rl_repo")

import os
import numpy as np
import ml_dtypes

import concourse.bass as bass
import concourse.bacc as bacc
import concourse.mybir as mybir
import concourse.tile as tile
from concourse.bass_utils import run_bass_kernel_spmd

F32 = mybir.dt.float32
BF16 = mybir.dt.bfloat16
I32 = mybir.dt.int32
AF = mybir.ActivationFunctionType

G, M, D, U = 8192, 32, 128, 200000
H_ATT = 16
NC = 8
GPC = G // NC          # 1024 groups per core
R = GPC * M            # 32768 member rows per core
NT = R // 128          # 256 tiles of 128 rows
NSB = GPC // 128       # 8 superblocks of 128 groups (32 tiles each)
KT = G // 128          # 64 k-tiles for the big matmuls
HGPC = GPC // 2        # half of the per-core groups (AllGather halves)

_CACHE = {}


def _build(gather="device"):
    nc = bacc.Bacc("TRN2", target_bir_lowering=False, debug=False)

    # ---- inputs ----
    if gather == "device":
        sum_tab = nc.dram_tensor("sum_tab", [U, D], BF16, kind="ExternalInput")
        gidx = nc.dram_tensor("gidx", [128, NT], I32, kind="ExternalInput")
    else:
        memb = nc.dram_tensor("memb", [128, NT * D], BF16, kind="ExternalInput")
    mdiag = nc.dram_tensor("mdiag", [128, NT * 4], BF16, kind="ExternalInput")
    gtabt = nc.dram_tensor("gtabt", [D, GPC], F32, kind="ExternalInput")
    ht = nc.dram_tensor("ht", [G, GPC], BF16, kind="ExternalInput")
    aw1 = nc.dram_tensor("aw1", [D, H_ATT], BF16, kind="ExternalInput")
    aw2r = nc.dram_tensor("aw2r", [128, 1], BF16, kind="ExternalInput")
    ab1s = nc.dram_tensor("ab1s", [128, 1], F32, kind="ExternalInput")
    ab2s = nc.dram_tensor("ab2s", [128, 1], F32, kind="ExternalInput")
    hw1 = nc.dram_tensor("hw1", [D, D], BF16, kind="ExternalInput")
    hw2 = nc.dram_tensor("hw2", [D, D], BF16, kind="ExternalInput")
    id_bf = nc.dram_tensor("id_bf", [128, 128], BF16, kind="ExternalInput")
    id_f32 = nc.dram_tensor("id_f32", [128, 128], F32, kind="ExternalInput")
    ind4 = nc.dram_tensor("ind4", [1, 512], F32, kind="ExternalInput")
    ones_col = nc.dram_tensor("ones_col", [128, 1], BF16, kind="ExternalInput")

    outt = nc.dram_tensor("outt", [D, GPC], F32, kind="ExternalOutput")

    # internal DRAM for collectives — per-core quarters, stored transposed-
    # tiled [128, 256] so the post-AllGather reload is 1KB-contiguous per
    # partition: vd[q][p, j*128+d] = V[local group q*256 + j*128 + p, d]
    QG = GPC // 4
    vd = [nc.dram_tensor(f"vd{q}", [128, QG], BF16) for q in range(4)]
    vag = [
        nc.dram_tensor(f"vag{q}", [NC * 128, QG], BF16, addr_space="Shared")
        for q in range(4)
    ]
    wd = [nc.dram_tensor(f"wd{q}", [128, QG], BF16) for q in range(4)]
    wag = [
        nc.dram_tensor(f"wag{q}", [NC * 128, QG], BF16, addr_space="Shared")
        for q in range(4)
    ]

    def allgather(src, dst):
        nc.gpsimd.collective_compute(
            "AllGather",
            mybir.AluOpType.bypass,
            replica_groups=[list(range(NC))],
            ins=[src[:]],
            outs=[dst[:]],
        )

    with tile.TileContext(nc) as tc:
        with (
            tc.tile_pool(name="const", bufs=1) as cpool,
            tc.tile_pool(name="htpool", bufs=1) as htpool,
            tc.tile_pool(name="gath", bufs=2) as gpool,
            tc.tile_pool(name="kagg", bufs=4) as kpool,
            tc.tile_pool(name="work", bufs=2) as wpool,
            tc.tile_pool(name="big", bufs=1) as xpool,
            tc.tile_pool(name="ps_e", bufs=2, space="PSUM") as pse,
            tc.tile_pool(name="ps_h", bufs=2, space="PSUM") as psh,
            tc.tile_pool(name="ps_m", bufs=2, space="PSUM") as psm,
            tc.tile_pool(name="ps_y", bufs=1, space="PSUM") as psy,
        ):
            # ---- constants ----
            aw1_sb = cpool.tile([D, H_ATT], BF16, tag="aw1")
            nc.sync.dma_start(aw1_sb[:], aw1[:])
            aw2r_sb = cpool.tile([128, 1], BF16, tag="aw2r")
            nc.sync.dma_start(aw2r_sb[:], aw2r[:])
            ab1s_sb = cpool.tile([128, 1], F32, tag="ab1s")
            nc.sync.dma_start(ab1s_sb[:], ab1s[:])
            ab2s_sb = cpool.tile([128, 1], F32, tag="ab2s")
            nc.sync.dma_start(ab2s_sb[:], ab2s[:])
            hw1_sb = cpool.tile([D, D], BF16, tag="hw1")
            nc.sync.dma_start(hw1_sb[:], hw1[:])
            hw2_sb = cpool.tile([D, D], BF16, tag="hw2")
            nc.sync.dma_start(hw2_sb[:], hw2[:])
            idbf_sb = cpool.tile([128, 128], BF16, tag="idbf")
            nc.sync.dma_start(idbf_sb[:], id_bf[:])
            idf_sb = cpool.tile([128, 128], F32, tag="idf")
            nc.sync.dma_start(idf_sb[:], id_f32[:])
            ind4_sb = cpool.tile([1, 512], F32, tag="ind4")
            nc.sync.dma_start(ind4_sb[:], ind4[:])
            ones_sb = cpool.tile([128, 1], BF16, tag="ones")
            nc.sync.dma_start(ones_sb[:], ones_col[:])
            mdiag_sb = cpool.tile([128, NT * 4], BF16, tag="mdiag")
            nc.sync.dma_start(mdiag_sb[:], mdiag[:])
            gtabt_sb = cpool.tile([D, GPC], F32, tag="gtabt")
            nc.sync.dma_start(gtabt_sb[:], gtabt[:])
            if gather == "device":
                gidx_sb = cpool.tile([128, NT], I32, tag="gidx")
                nc.sync.dma_start(gidx_sb[:], gidx[:])

            # ---- H^T resident [128, KT*GPC] bf16 (16MB) ----
            ht_sb = htpool.tile([128, KT * GPC], BF16, tag="ht")
            for k in range(KT):
                nc.sync.dma_start(
                    ht_sb[:, k * GPC : (k + 1) * GPC],
                    ht[k * 128 : (k + 1) * 128, :],
                )

            # ---- attention over 8 superblocks ----
            def fill_gbig(sbi, tile_):
                if gather == "device":
                    for k in range(32):
                        t = sbi * 32 + k
                        nc.gpsimd.indirect_dma_start(
                            out=tile_[:, k * 128 : (k + 1) * 128],
                            out_offset=None,
                            in_=sum_tab[:],
                            in_offset=bass.IndirectOffsetOnAxis(
                                ap=gidx_sb[:, t : t + 1], axis=0
                            ),
                        )
                else:
                    nc.scalar.dma_start(
                        tile_[:], memb[:, sbi * 32 * 128 : (sbi + 1) * 32 * 128]
                    )

            vag_sb = [None, None, None, None]
            gbig = gpool.tile([128, 32 * 128], BF16, tag="gbig", name="gbig0")
            fill_gbig(0, gbig)
            etbig = None
            for sb in range(NSB):
                if sb + 1 < NSB:
                    gbig_nxt = gpool.tile(
                        [128, 32 * 128], BF16, tag="gbig", name=f"gbig{sb + 1}"
                    )
                    fill_gbig(sb + 1, gbig_nxt)

                misc = psm.tile([128, 512], F32, tag="misc")
                logit_ps = misc[:, 0:32]
                den_ps = misc[:, 32:33]
                denbc_ps = misc[:, 36:68]
                dent_ps = misc[0:1, 68:196]
                gatt_ps = misc[:, 196:324]
                v_ps = misc[:, 324:452]

                # chunks of 512 rows; hid^T for 3 chunks stacked per PSUM
                # tile at partition offsets 0/32/64 (96 is not encodable)
                embT_sb = None
                hid_ps = None
                for c in range(8):
                    if c % 2 == 0:
                        embT_ps = pse.tile([128, 1024], BF16, tag="embT")
                        for l in range(2):
                            for s in range(4):
                                t = 4 * (c + l) + s
                                nc.tensor.transpose(
                                    embT_ps[:, l * 512 + s * 128 : l * 512 + (s + 1) * 128],
                                    gbig[:, t * 128 : (t + 1) * 128],
                                    idbf_sb[:],
                                )
                        embT_sb = wpool.tile([128, 1024], BF16, tag="embT_sb")
                        nc.vector.tensor_copy(embT_sb[:], embT_ps[:])
                    embT_src = embT_sb[:, (c % 2) * 512 : (c % 2 + 1) * 512]
                    q, j = c // 3, c % 3
                    if j == 0:
                        hid_ps = psh.tile([128, 512], F32, tag="hid")
                        hid_tiles = hid_ps
                    nc.tensor.matmul(
                        hid_tiles[32 * j : 32 * j + 16, :],
                        aw1_sb[:],
                        embT_src,
                        start=True,
                        stop=True,
                    )
                    if c in (2, 5, 7):
                        nj = 3 if c != 7 else 2
                        hidT_sb = wpool.tile([128, 512], BF16, tag="hidT")
                        nc.scalar.activation(
                            hidT_sb[:], hid_tiles[:], AF.Relu, bias=ab1s_sb[:, :1]
                        )
                        for jj in range(nj):
                            for s in range(4):
                                tl = 4 * (3 * q + jj) + s  # tile in superblock
                                nc.tensor.matmul(
                                    logit_ps[:, tl : tl + 1],
                                    hidT_sb[32 * jj : 32 * jj + 16, s * 128 : (s + 1) * 128],
                                    aw2r_sb[32 * jj : 32 * jj + 16, :],
                                    start=True,
                                    stop=True,
                                )

                p_sb = wpool.tile([128, 32], BF16, tag="p")
                nc.scalar.activation(p_sb[:], logit_ps, AF.Exp, bias=ab2s_sb[:, :1])
                pm_sb = wpool.tile([128, 128], BF16, tag="pm")
                nc.vector.tensor_tensor(
                    pm_sb[:].rearrange("p (t l) -> p t l", l=4),
                    p_sb[:].rearrange("p (t o) -> p t o", o=1).to_broadcast(
                        [128, 32, 4]
                    ),
                    mdiag_sb[:, sb * 128 : (sb + 1) * 128].rearrange(
                        "p (t l) -> p t l", l=4
                    ),
                    mybir.AluOpType.mult,
                )
                # denominators: den[col] = sum_rows pm[row, col]
                nc.tensor.matmul(den_ps, pm_sb[:], ones_sb[:], start=True, stop=True)
                den_sb = wpool.tile([128, 1], F32, tag="den")
                nc.vector.tensor_copy(den_sb[:], den_ps)
                nc.tensor.transpose(dent_ps, den_sb[:], idf_sb[:])
                dent_sb = wpool.tile([1, 128], F32, tag="dent")
                nc.vector.tensor_copy(dent_sb[:], dent_ps)
                # den_bc[r, t] = den[4t + r//32] via 4 K=1 matmuls
                dent_re = dent_sb[:].rearrange("p (t l) -> p t l", l=4)
                for gl in range(4):
                    nc.tensor.matmul(
                        denbc_ps,
                        ind4_sb[0:1, gl * 128 : (gl + 1) * 128],
                        dent_re[:, :, gl : gl + 1],
                        start=(gl == 0),
                        stop=(gl == 3),
                    )
                recip_sb = wpool.tile([128, 32], F32, tag="recip")
                nc.vector.reciprocal(recip_sb[:], denbc_ps)
                maskp_sb = wpool.tile([128, 128], BF16, tag="maskp")
                nc.vector.tensor_tensor(
                    maskp_sb[:].rearrange("p (t l) -> p t l", l=4),
                    recip_sb[:].rearrange("p (t o) -> p t o", o=1).to_broadcast(
                        [128, 32, 4]
                    ),
                    pm_sb[:].rearrange("p (t l) -> p t l", l=4),
                    mybir.AluOpType.mult,
                )
                # g_att^T accumulation: [128 d, 128 groups]
                for t in range(32):
                    nc.tensor.matmul(
                        gatt_ps[:, 4 * t : 4 * t + 4],
                        gbig[:, t * 128 : (t + 1) * 128],
                        maskp_sb[:, 4 * t : 4 * t + 4],
                        start=True,
                        stop=True,
                    )
                xt_sb = wpool.tile([128, 128], BF16, tag="xt")
                nc.vector.tensor_tensor(
                    xt_sb[:],
                    gatt_ps,
                    gtabt_sb[:, sb * 128 : (sb + 1) * 128],
                    mybir.AluOpType.add,
                )
                nc.tensor.matmul(v_ps, xt_sb[:], hw1_sb[:], start=True, stop=True)
                v_sb = wpool.tile([128, 128], BF16, tag="v")
                nc.vector.tensor_copy(v_sb[:], v_ps)
                nc.scalar.dma_start(
                    vd[sb // 2][:, (sb % 2) * 128 : (sb % 2 + 1) * 128], v_sb[:]
                )
                if sb % 2 == 1:
                    qq = sb // 2
                    allgather(vd[qq], vag[qq])
                    vag_sb[qq] = kpool.tile(
                        [128, NC * 256], BF16, tag="kq", name=f"vag_sb{qq}"
                    )
                    nc.sync.dma_start(
                        vag_sb[qq][:].rearrange("p (c f) -> p c f", f=256),
                        vag[qq].rearrange("(c p) f -> p c f", p=128),
                    )
                if sb + 1 < NSB:
                    gbig = gbig_nxt

            # ---- stage 1: Y^T = V^T H^T (k-split across AG halves) ----
            y_ps = [
                psy.tile([128, 512], F32, tag="y0", name="y_ps0"),
                psy.tile([128, 512], F32, tag="y1", name="y_ps1"),
            ]
            ht_all = xpool.tile([128, GPC], BF16, tag="hT")
            wag_sb = [None, None, None, None]
            # column-quarter-major: each W quarter + its AllGather launches
            # as soon as its 256 columns finish accumulating
            for wq in range(4):
                c2, hf = wq // 2, wq % 2
                for q in range(4):
                    for kk in range(16):
                        k = q * 16 + kk
                        nc.tensor.matmul(
                            y_ps[c2][:, hf * 256 : hf * 256 + 256],
                            vag_sb[q][:, kk * 128 : (kk + 1) * 128],
                            ht_sb[:, k * GPC + wq * 256 : k * GPC + wq * 256 + 256],
                            start=(k == 0),
                            stop=(k == KT - 1),
                        )
                nc.scalar.activation(
                    ht_all[:, wq * 256 : (wq + 1) * 256],
                    y_ps[c2][:, hf * 256 : hf * 256 + 256],
                    AF.Relu,
                )
                for gb in range(2):
                    g0 = wq * 2 + gb
                    wmisc = psm.tile([128, 512], F32, tag="misc")
                    w_ps = wmisc[:, 0:128]
                    nc.tensor.matmul(
                        w_ps,
                        ht_all[:, g0 * 128 : (g0 + 1) * 128],
                        hw2_sb[:],
                        start=True,
                        stop=True,
                    )
                    w_sb = wpool.tile([128, 128], BF16, tag="w")
                    nc.vector.tensor_copy(w_sb[:], w_ps)
                    nc.scalar.dma_start(
                        wd[wq][:, gb * 128 : (gb + 1) * 128], w_sb[:]
                    )
                allgather(wd[wq], wag[wq])
                wag_sb[wq] = kpool.tile(
                    [128, NC * 256], BF16, tag="kq", name=f"wag_sb{wq}"
                )
                nc.sync.dma_start(
                    wag_sb[wq][:].rearrange("p (c f) -> p c f", f=256),
                    wag[wq].rearrange("(c p) f -> p c f", p=128),
                )

            # ---- stage 2: out^T = W^T H^T ----
            o_ps = [
                psy.tile([128, 512], F32, tag="y0", name="o_ps0"),
                psy.tile([128, 512], F32, tag="y1", name="o_ps1"),
            ]
            for q in range(4):
                for c2 in range(2):
                    for kk in range(16):
                        k = q * 16 + kk
                        nc.tensor.matmul(
                            o_ps[c2][:],
                            wag_sb[q][:, kk * 128 : (kk + 1) * 128],
                            ht_sb[:, k * GPC + c2 * 512 : k * GPC + c2 * 512 + 512],
                            start=(k == 0),
                            stop=(k == KT - 1),
                        )
            for c2 in range(2):
                ot_sb = xpool.tile([128, 512], F32, tag=f"ot{c2}")
                nc.vector.tensor_copy(ot_sb[:], o_ps[c2][:])
                nc.sync.dma_start(outt[:, c2 * 512 : (c2 + 1) * 512], ot_sb[:])

    nc.compile()
    return nc


def _prep_inputs(group_inputs, members, member_mask, user_embedding, H_gl,
                 user_table, group_table, aw1, ab1, aw2, ab2, hw1, hw2,
                 gather="device"):
    bf = ml_dtypes.bfloat16
    sum_tab = (
        np.asarray(user_table, np.float32) + np.asarray(user_embedding, np.float32)
    )
    gi = np.asarray(group_inputs, np.int64)
    gtab_full = np.asarray(group_table, np.float32)[gi]
    Hg = np.asarray(H_gl, np.float32)

    aw2v = np.asarray(aw2, np.float32).reshape(-1)
    ab1v = np.asarray(ab1, np.float32).reshape(-1)
    aw2r = np.zeros((128, 1), np.float32)
    ab1s = np.zeros((128, 1), np.float32)
    for j in range(4):
        aw2r[32 * j : 32 * j + H_ATT, 0] = aw2v
        ab1s[32 * j : 32 * j + H_ATT, 0] = ab1v
    ab2s = np.full((128, 1), np.asarray(ab2, np.float32).reshape(-1)[0], np.float32)
    ind4 = np.zeros((1, 512), np.float32)
    for gl in range(4):
        ind4[0, gl * 128 + 32 * gl : gl * 128 + 32 * (gl + 1)] = 1.0

    # ht row-permutation matching the transposed-tiled AllGather layout:
    # k-tile (q, c, j) holds global groups c*GPC + q*256 + j*128 + p
    perm = np.concatenate(
        [
            np.arange(c * GPC + q * 256 + j * 128, c * GPC + q * 256 + (j + 1) * 128)
            for q in range(4)
            for c in range(NC)
            for j in range(2)
        ]
    )

    consts = dict(
        aw1=np.asarray(aw1, np.float32).astype(bf),
        aw2r=aw2r.astype(bf),
        ab1s=ab1s,
        ab2s=ab2s,
        hw1=np.asarray(hw1, np.float32).astype(bf),
        hw2=np.asarray(hw2, np.float32).astype(bf),
        id_bf=np.eye(128, dtype=np.float32).astype(bf),
        id_f32=np.eye(128, dtype=np.float32),
        ind4=ind4,
        ones_col=np.ones((128, 1), np.float32).astype(bf),
    )
    if gather == "device":
        consts["sum_tab"] = sum_tab.astype(bf)

    p = np.arange(128)
    gl_p = p // 32
    m_p = p % 32
    t_idx = np.arange(NT)
    in_maps = []
    for c in range(NC):
        sl = slice(c * GPC, (c + 1) * GPC)
        mem = np.asarray(members, np.int64)[sl].astype(np.int32).reshape(-1)
        mask01 = (np.asarray(member_mask, np.float32)[sl] > 0).astype(np.float32)
        val = mask01[(4 * t_idx[None, :] + gl_p[:, None]), m_p[:, None]]  # [128, NT]
        mdiag = np.zeros((128, NT, 4), np.float32)
        mdiag[p, :, gl_p] = val
        m = dict(
            consts,
            mdiag=np.ascontiguousarray(mdiag.reshape(128, NT * 4)).astype(bf),
            gtabt=np.ascontiguousarray(gtab_full[sl].T),
            ht=np.ascontiguousarray(Hg[sl].T[perm]).astype(bf),
        )
        if gather == "device":
            m["gidx"] = np.ascontiguousarray(mem.reshape(NT, 128).T)
        else:
            mb = sum_tab[mem.reshape(NT, 128)].astype(bf)  # [NT, 128, D]
            m["memb"] = np.ascontiguousarray(
                mb.transpose(1, 0, 2)
            ).reshape(128, NT * D)
        in_maps.append(m)
    return in_maps


def kernel(**inputs):
    gather = os.environ.get("KGATHER", "device")
    key = f"nc_{gather}"
    if key not in _CACHE:
        _CACHE[key] = _build(gather)
        _CACHE["nc"] = _CACHE[key]
    nc = _CACHE[key]
    in_maps = _prep_inputs(**inputs, gather=gather)
    res = run_bass_kernel_spmd(nc, in_maps, core_ids=list(range(NC)))
    out = np.concatenate(
        [np.ascontiguousarray(res.results[c]["outt"].T) for c in range(NC)], axis=0
    )
    return out.astype(np.float32)


if __name__ == "__main__":
    import reference
    inp = {k: np.asarray(v) for k, v in reference.setup_inputs().items()}
    exp = np.asarray(reference.reference(**inp))
    got = kernel(**inp)
    err = np.abs(got - exp).max() / (np.abs(exp).max() + 1e-30)
    rel = np.linalg.norm(got - exp) / (np.linalg.norm(exp) + 1e-30)
    print(f"absmax-rel: {err:.3e}  fro-rel: {rel:.3e}")
